# revision 31
# baseline (speedup 1.0000x reference)
"""Trainium2 Bass kernel for a dense pre-LN transformer block.

Reference computation (B=2, T=2048, E=1024, H=16, HS=64):
    h  = LN(x; g1, be1)
    q, k, v = per-head projections of h        (wq/wk/wv: [H, E, HS])
    att = causal softmax(q k^T / sqrt(E)) v    (per head)
    x2 = x + concat(att) @ w_proj + b_proj
    y  = x2 + relu(LN(x2; g2, be2) @ w1 + b1) @ w2 + b2

Distribution over 8 NeuronCores:
  - QKV + attention: tensor-parallel over heads (2 heads/core).
  - proj + FFN: data-parallel over token rows (512 tokens/core).
  - One AllToAll (attn output, feature-major) glues the two.

Device-side layout is feature-major ("transposed"): activations are
[feature, token]; the host pre-transposes x and pre-arranges weights so
the kernel never transposes activations on device.

LayerNorm is folded into the following matmul:
    LN(x) @ W = r_t * (x @ (g*W) - mu_t * colsum(g*W) + sigma_t * (be@W))
applied via two rank-1 "augmentation" rows in the contraction, where
mu/sigma/r are per-token stats computed with ones-matmuls (bf16, exact
to ~1e-4). Softmax runs without max-subtraction (logits are bounded by
design: scale 1/sqrt(E)=1/32); the per-token r for V is folded into the
exp() bias as ln(r_s), and the softmax denominator is obtained from a
sigma-column appended to V in the PV matmul.
"""

import sys
from contextlib import ExitStack
import numpy as np

sys.path.insert(0, "/opt/trn_rl_repo")

# ---------------------------------------------------------------- constants
B, T, E, H = 2, 2048, 1024, 16
HS = E // H          # 64
W = 8                # cores
BT = B * T           # 4096 tokens total
TB = BT // W         # 512 tokens per core (row-DP block)
HPC = H // W         # 2 heads per core
FCH = E // 128       # 8 feature chunks
M4E = 4 * E          # 4096 ffn hidden
MCH = M4E // 128     # 32 hidden chunks
TT = 512             # token tile (matmul moving dim)
NTT = BT // TT       # 8 token tiles
SCH = BT // 128      # 32 token chunks of 128 (for v / s-chunks)
EPS = 1e-5

# dtype knobs: "f32" or "bf16"
import os as _os
MM_DT_S = _os.environ.get("KMM_DT", "bf16")    # linear-layer matmuls
ATT_DT_S = _os.environ.get("KATT_DT", "bf16")  # attention matmuls

_CACHE = {}


def _build(nc, mm_dt, att_dt, f32):
    import concourse.bass as bass
    from concourse.tile import TileContext
    import concourse.mybir as mybir

    AF = mybir.ActivationFunctionType
    dp = nc.declare_dram_parameter

    use_f32_x = mm_dt == f32  # stream fp32 x for linear matmuls

    # ------------------------------------------------- DRAM parameters
    if use_f32_x:
        xT_d = dp("xT", [FCH, 128, BT], f32, isOutput=False)
    xh_d = dp("xh", [FCH, 128, BT], mybir.dt.bfloat16, isOutput=False)
    xtb_d = dp("xtb", [FCH, 128, TB], f32, isOutput=False)
    wqkv_d = dp("wqkv", [FCH, 128, 3 * 128], mm_dt, isOutput=False)
    augw_d = dp("augw", [2, 3 * 128], mm_dt, isOutput=False)
    wproj_d = dp("wproj", [FCH, 128, E], mm_dt, isOutput=False)
    bproj_d = dp("bproj", [128, FCH], f32, isOutput=False)
    w1_d = dp("w1", [FCH, 128, M4E], mm_dt, isOutput=False)  # [s][p][(o,c512)] layout
    augw1_d = dp("augw1", [2, M4E], mm_dt, isOutput=False)
    w2_d = dp("w2", [FCH, 128, M4E], mm_dt, isOutput=False)  # [et,p,(mc,f)] host layout
    b2_d = dp("b2c", [128, FCH], f32, isOutput=False)
    mask_d = dp("mask", [4, 128, TT], att_dt, isOutput=False)
    y_d = dp("y", [FCH, 128, TB], f32, isOutput=True)


    es = ExitStack()
    with TileContext(nc) as tc, es:
        # ------------------------------------------------- pools
        glob = es.enter_context(tc.tile_pool(name="glob", bufs=1))
        dramp = es.enter_context(tc.tile_pool(name="dramp", bufs=1, space="DRAM"))
        a2a_in = [dramp.tile([W, 64, TB], mm_dt, tag=f"a2a_in{h}", name=f"a2a_in{h}") for h in (0, 1)]
        a2a_out = [dramp.tile([W, 64, TB], mm_dt, tag=f"a2a_out{h}", name=f"a2a_out{h}") for h in (0, 1)]
        psb = es.enter_context(tc.tile_pool(name="psb", bufs=2, space="PSUM"))
        pss = es.enter_context(tc.tile_pool(name="pss", bufs=2, space="PSUM"))
        ps2 = es.enter_context(tc.tile_pool(name="ps2", bufs=2, space="PSUM"))

        ones_bf = glob.tile([128, 1], mybir.dt.bfloat16, tag="ones_bf")
        nc.vector.memset(ones_bf[:], 1.0)
        ones_f1 = glob.tile([1, 1], f32, tag="ones_f1")
        nc.vector.memset(ones_f1[:], 1.0)

        eps1 = glob.tile([1, 1], f32, tag="eps1")
        nc.vector.memset(eps1[:], EPS)
        attn_slab = glob.tile([128, BT], mm_dt, tag="attn_slab")

        # ========================================================= PHASE A
        es_a = es.enter_context(ExitStack())
        pha = es_a.enter_context(tc.tile_pool(name="pha", bufs=1))
        str_a = es_a.enter_context(tc.tile_pool(name="stra", bufs=2))
        expp = es_a.enter_context(
            tc.tile_pool(name="expp", bufs=18 if mm_dt != f32 else 10))
        finp = es_a.enter_context(tc.tile_pool(name="finp", bufs=3))

        wqkv = pha.tile([128, FCH, 3 * 128], mm_dt, tag="wqkv")
        for o in range(FCH):
            nc.sync.dma_start(wqkv[:, o, :], wqkv_d[o])
        augw = pha.tile([2, 3 * 128], mm_dt, tag="augw")
        nc.sync.dma_start(augw[:], augw_d[:])
        masks = pha.tile([128, 4, TT], att_dt, tag="masks")
        for kk in range(4):
            nc.sync.dma_start(masks[:, kk, :], mask_d[kk])

        aug_all = pha.tile([2, BT], mm_dt, tag="aug_all")
        r_ball = pha.tile([128, BT], mm_dt, tag="r_ball")
        qT = pha.tile([128, BT], att_dt, tag="qT")
        kT = pha.tile([128, BT], att_dt, tag="kT")
        v_slab = pha.tile([128, SCH, 130], att_dt, tag="v_slab")
        nc.vector.memset(v_slab[:], 1.0)
        r_cols = pha.tile([128, SCH], f32, tag="r_cols")

        # resident bf16 x slab (stats always; matmul rhs in bf16 mode)
        xhs = pha.tile([128, FCH, BT], mybir.dt.bfloat16, tag="xhs")
        for tj in range(NTT):
            for o in range(FCH):
                nc.sync.dma_start(xhs[:, o, tj * TT:(tj + 1) * TT],
                                  xh_d[o, :, tj * TT:(tj + 1) * TT])

        # ---- A1: all token stats (incl. r/sigma columns), pipelined
        for tj in range(NTT):
            tsl = slice(tj * TT, (tj + 1) * TT)
            ps_sum = pss.tile([1, TT], f32, tag="small")
            ps_sq = pss.tile([1, TT], f32, tag="small")
            for o in range(FCH):
                sq = str_a.tile([128, TT], mybir.dt.bfloat16, tag="sq")
                nc.vector.tensor_mul(sq[:], xhs[:, o, tsl], xhs[:, o, tsl])
                nc.tensor.matmul(ps_sum[:], ones_bf[:], xhs[:, o, tsl],
                                 start=(o == 0), stop=(o == FCH - 1))
                nc.tensor.matmul(ps_sq[:], ones_bf[:], sq[:],
                                 start=(o == 0), stop=(o == FCH - 1))
            mu_row = str_a.tile([1, TT], f32, tag="mu_row")
            nc.vector.tensor_scalar_mul(mu_row[:], ps_sum[:], 1.0 / E)
            msq_row = str_a.tile([1, TT], f32, tag="msq_row")
            nc.vector.tensor_scalar_mul(msq_row[:], ps_sq[:], 1.0 / E)
            var_row = str_a.tile([1, TT], f32, tag="var_row")
            nc.vector.tensor_mul(var_row[:], mu_row[:], mu_row[:])
            nc.vector.tensor_sub(var_row[:], msq_row[:], var_row[:])
            sig_row = str_a.tile([1, TT], f32, tag="sig_row")
            nc.scalar.activation(sig_row[:], var_row[:], AF.Sqrt, bias=eps1[:])
            r_row = str_a.tile([1, TT], f32, tag="r_row")
            nc.vector.reciprocal(r_row[:], sig_row[:])
            nc.vector.tensor_copy(aug_all[0:1, tsl], mu_row[:])
            nc.gpsimd.dma_start(aug_all[1:2, tsl], sig_row[:])
            r_row_mm = str_a.tile([1, TT], mm_dt, tag="r_row_mm")
            nc.vector.tensor_copy(r_row_mm[:], r_row[:])
            nc.gpsimd.partition_broadcast(r_ball[:, tsl], r_row_mm[:])
            for j in range(4):
                g = tj * 4 + j
                csl = slice(j * 128, (j + 1) * 128)
                pr = pss.tile([128, 1], f32, tag="small")
                nc.tensor.matmul(pr[:], r_row[0:1, csl], ones_f1[:])
                nc.vector.tensor_copy(r_cols[:, g:g + 1], pr[:])

        # ---- A2: dense QKV
        for tj in range(NTT):
            tsl = slice(tj * TT, (tj + 1) * TT)
            if use_f32_x:
                xt = str_a.tile([128, FCH, TT], f32, tag="xt")
                for o in range(FCH):
                    nc.sync.dma_start(xt[:, o, :], xT_d[o, :, tsl])
                x_mm = xt
                xsl = slice(0, TT)
            else:
                x_mm = xhs
                xsl = tsl
            for fg, slab in ((0, qT), (1, kT)):
                fsl = slice(fg * 128, (fg + 1) * 128)
                ps = psb.tile([128, TT], f32, tag="big")
                for o in range(FCH):
                    nc.tensor.matmul(ps[:], wqkv[:, o, fsl], x_mm[:, o, xsl],
                                     start=(o == 0), stop=False)
                nc.tensor.matmul(ps[:], augw[:, fsl], aug_all[:, tsl],
                                 start=False, stop=True)
                nc.vector.tensor_mul(slab[:, tsl], ps[:], r_ball[:, tsl])
            for j in range(4):
                g = tj * 4 + j
                csl = slice(xsl.start + j * 128, xsl.start + (j + 1) * 128)
                ps = psb.tile([128, 128], f32, tag="big")
                for o in range(FCH):
                    nc.tensor.matmul(ps[:], x_mm[:, o, csl],
                                     wqkv[:, o, 256:384],
                                     start=(o == 0), stop=False)
                nc.tensor.matmul(ps[:], aug_all[:, g * 128:(g + 1) * 128],
                                 augw[:, 256:384],
                                 start=False, stop=True)
                nc.vector.tensor_scalar_mul(v_slab[:, g, 0:64], ps[:, 0:64],
                                             r_cols[:, g:g + 1])
                nc.vector.tensor_scalar_mul(v_slab[:, g, 65:129], ps[:, 64:128],
                                            r_cols[:, g:g + 1])

        # ---- A3: attention, software-pipelined one tile deep: in each
        # round, PV matmuls of the previous tile interleave with the score
        # matmuls of the current tile, so PE never waits on exp latency.
        def scores_thunks(hh, b, tj):
            hsl = slice(hh * 64, (hh + 1) * 64)
            tsl = slice(b * T + tj * TT, b * T + (tj + 1) * TT)
            n_si = 4 * tj + 4
            exs = []
            thunks = []
            state = {}

            def mk_mm(si, half):
                def f():
                    g = b * (T // 128) + si
                    ssl = slice(g * 128, (g + 1) * 128)
                    if half == 0:
                        state['ps'] = ps2.tile([128, 2, TT], f32, tag="big2", name="ps_pair")
                    ps = state['ps']
                    nc.tensor.matmul(ps[:, half, :], kT[hsl, ssl],
                                     qT[hsl, tsl], start=True, stop=True)
                return f

            def mk_exp(si_pair):
                def f():
                    ps = state['ps']
                    ex = expp.tile([128, 2, TT], att_dt, tag="ex")
                    nc.scalar.activation(ex[:], ps[:], AF.Exp,
                                         scale=float(E) ** -0.5)
                    for half, si in enumerate(si_pair):
                        exv = ex[:, half, :]
                        if si >= 4 * tj:
                            nc.vector.tensor_mul(exv, exv,
                                                 masks[:, si - 4 * tj, :])
                        g = b * (T // 128) + si
                        exs.append((g, exv))
                return f

            for si in range(0, n_si, 2):
                thunks.append(mk_mm(si, 0))
                thunks.append(mk_mm(si + 1, 1))
                thunks.append(mk_exp((si, si + 1)))
            return (hh, b, tj, exs, thunks)

        def pv_thunks(pend):
            hh, b, tj, exs, _ = pend
            hsl = slice(hh * 64, (hh + 1) * 64)
            tsl = slice(b * T + tj * TT, b * T + (tj + 1) * TT)
            vof = 65 * hh
            n = 4 * tj + 4
            po = psb.tile([65, TT], f32, tag="big")
            thunks = []

            def mk(idx):
                def f():
                    g, ex = exs[idx]
                    nc.tensor.matmul(po[:], v_slab[:, g, vof:vof + 65],
                                     ex, start=(idx == 0),
                                     stop=(idx == n - 1))
                return f

            for idx in range(n):
                thunks.append(mk(idx))

            def fin():
                rd = finp.tile([1, TT], f32, tag="rd")
                nc.vector.reciprocal(rd[:], po[64:65, :])
                rb = finp.tile([64, TT], f32, tag="rb")
                nc.gpsimd.partition_broadcast(rb[:], rd[:])
                nc.vector.tensor_mul(attn_slab[hsl, tsl], po[0:64, :], rb[:])
                d = b * (T // TT) + tj
                nc.sync.dma_start(a2a_in[hh][d],
                                  attn_slab[hsl, d * TB:(d + 1) * TB])
            thunks.append(fin)
            return thunks

        for hh in (0, 1):
            pend = None
            for b in range(B):
                for tj in range(T // TT):
                    cur = scores_thunks(hh, b, tj)
                    pv = pv_thunks(pend) if pend is not None else []
                    sc = cur[4]
                    for i in range(max(len(pv), len(sc))):
                        if i < len(pv):
                            pv[i]()
                        if i < len(sc):
                            sc[i]()
                    pend = cur
            for t in pv_thunks(pend):
                t()
            nc.gpsimd.collective_compute(
                "AllToAll", mybir.AluOpType.bypass,
                ins=[a2a_in[hh].opt()], outs=[a2a_out[hh].opt()],
                replica_groups=[list(range(W))],
            )

        es_a.close()

        # ========================================================= PHASE B
        phb = es.enter_context(tc.tile_pool(name="phb", bufs=1))
        str_b = es.enter_context(tc.tile_pool(name="strb", bufs=2))

        atf = phb.tile([128, FCH, TB], mm_dt, tag="atf")
        for o in range(FCH):
            nc.sync.dma_start(atf[0:64, o, :], a2a_out[0][o])
            nc.sync.dma_start(atf[64:128, o, :], a2a_out[1][o])
        xtb = phb.tile([128, FCH, TB], f32, tag="xtb")
        for o in range(FCH):
            nc.sync.dma_start(xtb[:, o, :], xtb_d[o])
        bproj = phb.tile([128, FCH], f32, tag="bproj")
        nc.sync.dma_start(bproj[:], bproj_d[:])
        b2c = phb.tile([128, FCH], f32, tag="b2c")
        nc.sync.dma_start(b2c[:], b2_d[:])
        augw1 = phb.tile([2, M4E], mm_dt, tag="augw1")
        nc.sync.dma_start(augw1[:], augw1_d[:])

        # ---- proj + residual -> x2T
        x2T = phb.tile([128, FCH, TB], f32, tag="x2T")
        for et in range(FCH):
            esl = slice(et * 128, (et + 1) * 128)
            wp = str_b.tile([128, FCH, 128], mm_dt, tag="wp")
            nc.sync.dma_start(wp[:], wproj_d[et].rearrange("p (o c) -> p o c", c=128))
            ps = psb.tile([128, TB], f32, tag="big")
            for o in range(FCH):
                nc.tensor.matmul(ps[:], wp[:, o, :], atf[:, o, :],
                                 start=(o == 0), stop=(o == FCH - 1))
            nc.vector.scalar_tensor_tensor(
                x2T[:, et, :], ps[:], bproj[:, et:et + 1], xtb[:, et, :],
                mybir.AluOpType.add, mybir.AluOpType.add)

        # ---- LN2 stats
        xh2 = phb.tile([128, FCH, TB], mybir.dt.bfloat16, tag="xh2")
        ps_sum = pss.tile([1, TB], f32, tag="small")
        ps_sq = pss.tile([1, TB], f32, tag="small")
        for o in range(FCH):
            nc.vector.tensor_copy(xh2[:, o, :], x2T[:, o, :])
            sq = str_b.tile([128, TB], mybir.dt.bfloat16, tag="sq2")
            nc.vector.tensor_mul(sq[:], xh2[:, o, :], xh2[:, o, :])
            nc.tensor.matmul(ps_sum[:], ones_bf[:], xh2[:, o, :],
                             start=(o == 0), stop=(o == FCH - 1))
            nc.tensor.matmul(ps_sq[:], ones_bf[:], sq[:],
                             start=(o == 0), stop=(o == FCH - 1))
        mu2 = phb.tile([1, TB], f32, tag="mu2")
        nc.vector.tensor_scalar_mul(mu2[:], ps_sum[:], 1.0 / E)
        msq2 = phb.tile([1, TB], f32, tag="msq2")
        nc.vector.tensor_scalar_mul(msq2[:], ps_sq[:], 1.0 / E)
        var2 = phb.tile([1, TB], f32, tag="var2")
        nc.vector.tensor_mul(var2[:], mu2[:], mu2[:])
        nc.vector.tensor_sub(var2[:], msq2[:], var2[:])
        sig2 = phb.tile([1, TB], f32, tag="sig2")
        nc.scalar.activation(sig2[:], var2[:], AF.Sqrt, bias=eps1[:])
        r2 = phb.tile([1, TB], f32, tag="r2")
        nc.vector.reciprocal(r2[:], sig2[:])
        aug2 = phb.tile([2, TB], mm_dt, tag="aug2")
        nc.vector.tensor_copy(aug2[0:1, :], mu2[:])
        nc.gpsimd.dma_start(aug2[1:2, :], sig2[:])
        r2_b = phb.tile([128, TB], f32, tag="r2_b")
        nc.gpsimd.partition_broadcast(r2_b[:], r2[:])

        x2mm = x2T if use_f32_x else xh2

        # ---- FFN1 -> relu slab (r2 deferred to FFN2 output: r2>0)
        relu = phb.tile([128, MCH, TB], mm_dt, tag="relu")
        for st in range(FCH):
            w1t = str_b.tile([128, FCH, 512], mm_dt, tag="w1t")
            nc.sync.dma_start(w1t[:], w1_d[st].rearrange("p (o c) -> p o c", c=512))
            for mj in range(4):
                mt = st * 4 + mj
                msl = slice(mt * 128, (mt + 1) * 128)
                jsl = slice(mj * 128, (mj + 1) * 128)
                ps = psb.tile([128, TB], f32, tag="big")
                for o in range(FCH):
                    nc.tensor.matmul(ps[:], w1t[:, o, jsl], x2mm[:, o, :],
                                     start=(o == 0), stop=False)
                nc.tensor.matmul(ps[:], augw1[:, msl], aug2[:],
                                 start=False, stop=True)
                nc.scalar.activation(relu[:, mt, :], ps[:], AF.Relu)

        # ---- FFN2 + r2 + residual + b2 -> y
        for et in range(FCH):
            ps = psb.tile([128, TB], f32, tag="big")
            for qq in range(4):
                w2t = str_b.tile([128, MCH // 4, 128], mm_dt, tag="w2t")
                nc.sync.dma_start(
                    w2t[:],
                    w2_d[et, :, qq * (M4E // 4):(qq + 1) * (M4E // 4)]
                    .rearrange("p (m f) -> p m f", f=128))
                for mj in range(MCH // 4):
                    mc = qq * (MCH // 4) + mj
                    nc.tensor.matmul(ps[:], w2t[:, mj, :], relu[:, mc, :],
                                     start=(mc == 0), stop=(mc == MCH - 1))
            u = str_b.tile([128, TB], f32, tag="u")
            nc.vector.tensor_mul(u[:], ps[:], r2_b[:])
            yt = str_b.tile([128, TB], f32, tag="yt")
            nc.vector.scalar_tensor_tensor(
                yt[:], u[:], b2c[:, et:et + 1], x2T[:, et, :],
                mybir.AluOpType.add, mybir.AluOpType.add)
            nc.sync.dma_start(y_d[et], yt[:])


    nc.finalize()
    return nc


def _get_nc():
    key = (MM_DT_S, ATT_DT_S)
    if key in _CACHE:
        return _CACHE[key]
    from concourse import bacc
    import concourse.mybir as mybir

    f32 = mybir.dt.float32
    mm_dt = f32 if MM_DT_S == "f32" else mybir.dt.bfloat16
    att_dt = f32 if ATT_DT_S == "f32" else mybir.dt.bfloat16
    nc = bacc.Bacc("TRN2", target_bir_lowering=False, debug=False,
                   num_devices=W)
    _build(nc, mm_dt, att_dt, f32)
    _CACHE[key] = nc
    return nc


def _prep_inputs(x, wq, wk, wv, w_proj, b_proj, w1, b1, w2, b2, g1, be1, g2, be2):
    """Host-side sharding: returns in_maps (list of 8 dicts)."""
    import ml_dtypes

    bf16 = ml_dtypes.bfloat16
    mm_np = np.float32 if MM_DT_S == "f32" else bf16
    att_np = np.float32 if ATT_DT_S == "f32" else bf16

    xf = np.ascontiguousarray(x.reshape(BT, E).T)          # [E, BT]
    xT = xf.reshape(FCH, 128, BT)
    xh = xT.astype(bf16)

    # causal mask tiles for the 4 diagonal-crossing offsets
    mask = np.zeros((4, 128, TT), dtype=att_np)
    uu = np.arange(TT)[None, :]
    pp = np.arange(128)[:, None]
    for k in range(4):
        mask[k] = (pp <= uu - 128 * k).astype(att_np)

    # [et][p][(o, c128)]: wproj_l[et, p, o*128+c] = w_proj[o*128+p, et*128+c]
    wpr = w_proj.reshape(FCH, 128, FCH, 128)                # [o, p, et, c]
    wproj_l = np.ascontiguousarray(wpr.transpose(2, 1, 0, 3).reshape(FCH, 128, E)).astype(mm_np)
    bproj_l = np.ascontiguousarray(b_proj.reshape(FCH, 128).T)  # [128, FCH]

    w1s = (g2[:, None] * w1)                                # [E, 4E]
    # [s][p][(o, c512)]: w1_l[s, p, o*512+c] = w1s[o*128+p, s*512+c]
    w1r = w1s.reshape(FCH, 128, FCH, 512)                   # [o, p, s, c]
    w1_l = np.ascontiguousarray(w1r.transpose(2, 1, 0, 3).reshape(FCH, 128, M4E)).astype(mm_np)
    aug1 = np.stack([-w1s.sum(axis=0), be2 @ w1 + b1]).astype(mm_np)

    # w2 host layout: [et, p, (mc, f)] with w2_l[et, p, mc*128+f] = w2[mc*128+p, et*128+f]
    w2r = w2.reshape(MCH, 128, FCH, 128)                    # [mc, p, et, f]
    w2_l = np.ascontiguousarray(w2r.transpose(2, 1, 0, 3).reshape(FCH, 128, M4E)).astype(mm_np)
    b2_l = np.ascontiguousarray(b2.reshape(FCH, 128).T)

    in_maps = []
    for c in range(W):
        hsl = slice(HPC * c, HPC * (c + 1))
        wq_c = wq[hsl].transpose(1, 0, 2).reshape(E, 128)
        wk_c = wk[hsl].transpose(1, 0, 2).reshape(E, 128)
        wv_c = wv[hsl].transpose(1, 0, 2).reshape(E, 128)
        wqkv = np.concatenate([g1[:, None] * wq_c,
                               g1[:, None] * wk_c,
                               g1[:, None] * wv_c], axis=1)  # [E, 384]
        augw = np.stack([-wqkv.sum(axis=0),
                         np.concatenate([be1 @ wq_c, be1 @ wk_c, be1 @ wv_c])]
                        ).astype(mm_np)
        m = {
            "xh": xh,
            "xtb": np.ascontiguousarray(xT[:, :, TB * c:TB * (c + 1)]),
            "wqkv": np.ascontiguousarray(wqkv.reshape(FCH, 128, 384)).astype(mm_np),
            "augw": augw,
            "wproj": wproj_l,
            "bproj": np.ascontiguousarray(bproj_l),
            "w1": w1_l,
            "augw1": aug1,
            "w2": w2_l,
            "b2c": np.ascontiguousarray(b2_l),
            "mask": mask,
        }
        if MM_DT_S == "f32":
            m["xT"] = xT
        in_maps.append(m)
    return in_maps


def kernel(**inputs):
    from concourse.bass_utils import run_bass_kernel_spmd

    nc = _get_nc()
    in_maps = _prep_inputs(**{k: np.asarray(v) for k, v in inputs.items()})
    res = run_bass_kernel_spmd(nc, in_maps, list(range(W)))
    # gather: core c produced y = [FCH, 128, TB] = yT block for tokens [TB*c, TB*(c+1))
    out_T = np.concatenate([res.results[c]["y"].reshape(E, TB)
                            for c in range(W)], axis=1)      # [E, BT]
    return np.ascontiguousarray(out_T.T).reshape(B, T, E).astype(np.float32)


# revision 32
# speedup vs baseline: 1.0213x; 1.0213x over previous
"""Trainium2 Bass kernel for a dense pre-LN transformer block.

Reference computation (B=2, T=2048, E=1024, H=16, HS=64):
    h  = LN(x; g1, be1)
    q, k, v = per-head projections of h        (wq/wk/wv: [H, E, HS])
    att = causal softmax(q k^T / sqrt(E)) v    (per head)
    x2 = x + concat(att) @ w_proj + b_proj
    y  = x2 + relu(LN(x2; g2, be2) @ w1 + b1) @ w2 + b2

Distribution over 8 NeuronCores:
  - QKV + attention: tensor-parallel over heads (2 heads/core).
  - proj + FFN: data-parallel over token rows (512 tokens/core).
  - One AllToAll (attn output, feature-major) glues the two.

Device-side layout is feature-major ("transposed"): activations are
[feature, token]; the host pre-transposes x and pre-arranges weights so
the kernel never transposes activations on device.

LayerNorm is folded into the following matmul:
    LN(x) @ W = r_t * (x @ (g*W) - mu_t * colsum(g*W) + sigma_t * (be@W))
applied via two rank-1 "augmentation" rows in the contraction, where
mu/sigma/r are per-token stats computed with ones-matmuls (bf16, exact
to ~1e-4). Softmax runs without max-subtraction (logits are bounded by
design: scale 1/sqrt(E)=1/32); the per-token r for V is folded into the
exp() bias as ln(r_s), and the softmax denominator is obtained from a
sigma-column appended to V in the PV matmul.
"""

import sys
from contextlib import ExitStack
import numpy as np

sys.path.insert(0, "/opt/trn_rl_repo")

# ---------------------------------------------------------------- constants
B, T, E, H = 2, 2048, 1024, 16
HS = E // H          # 64
W = 8                # cores
BT = B * T           # 4096 tokens total
TB = BT // W         # 512 tokens per core (row-DP block)
HPC = H // W         # 2 heads per core
FCH = E // 128       # 8 feature chunks
M4E = 4 * E          # 4096 ffn hidden
MCH = M4E // 128     # 32 hidden chunks
TT = 512             # token tile (matmul moving dim)
NTT = BT // TT       # 8 token tiles
SCH = BT // 128      # 32 token chunks of 128 (for v / s-chunks)
EPS = 1e-5

# dtype knobs: "f32" or "bf16"
import os as _os
MM_DT_S = _os.environ.get("KMM_DT", "bf16")    # linear-layer matmuls
ATT_DT_S = _os.environ.get("KATT_DT", "bf16")  # attention matmuls

_CACHE = {}


def _build(nc, mm_dt, att_dt, f32):
    import concourse.bass as bass
    from concourse.tile import TileContext
    import concourse.mybir as mybir

    AF = mybir.ActivationFunctionType
    dp = nc.declare_dram_parameter

    use_f32_x = mm_dt == f32  # stream fp32 x for linear matmuls

    # ------------------------------------------------- DRAM parameters
    if use_f32_x:
        xT_d = dp("xT", [FCH, 128, BT], f32, isOutput=False)
    xh_d = dp("xh", [FCH, 128, BT], mybir.dt.bfloat16, isOutput=False)
    xtb_d = dp("xtb", [FCH, 128, TB], f32, isOutput=False)
    wqkv_d = dp("wqkv", [FCH, 128, 3 * 128], mm_dt, isOutput=False)
    augw_d = dp("augw", [2, 3 * 128], mm_dt, isOutput=False)
    wproj_d = dp("wproj", [FCH, 128, E], mm_dt, isOutput=False)
    bproj_d = dp("bproj", [128, FCH], f32, isOutput=False)
    w1_d = dp("w1", [FCH, 128, M4E], mm_dt, isOutput=False)  # [s][p][(o,c512)] layout
    augw1_d = dp("augw1", [2, M4E], mm_dt, isOutput=False)
    w2_d = dp("w2", [FCH, 128, M4E], mm_dt, isOutput=False)  # [et,p,(mc,f)] host layout
    b2_d = dp("b2c", [128, FCH], f32, isOutput=False)
    mask_d = dp("mask", [4, 128, TT], att_dt, isOutput=False)
    y_d = dp("y", [FCH, 128, TB], f32, isOutput=True)


    es = ExitStack()
    with TileContext(nc) as tc, es:
        # ------------------------------------------------- pools
        glob = es.enter_context(tc.tile_pool(name="glob", bufs=1))
        dramp = es.enter_context(tc.tile_pool(name="dramp", bufs=1, space="DRAM"))
        a2a_in = [dramp.tile([W, 64, TB], mm_dt, tag=f"a2a_in{h}", name=f"a2a_in{h}") for h in (0, 1)]
        a2a_out = [dramp.tile([W, 64, TB], mm_dt, tag=f"a2a_out{h}", name=f"a2a_out{h}") for h in (0, 1)]


        ones_bf = glob.tile([128, 1], mybir.dt.bfloat16, tag="ones_bf")
        nc.vector.memset(ones_bf[:], 1.0)
        ones_f1 = glob.tile([1, 1], f32, tag="ones_f1")
        nc.vector.memset(ones_f1[:], 1.0)

        eps1 = glob.tile([1, 1], f32, tag="eps1")
        nc.vector.memset(eps1[:], EPS)
        attn_slab = glob.tile([128, BT], mm_dt, tag="attn_slab")

        # ========================================================= PHASE A
        es_a = es.enter_context(ExitStack())
        es_a12 = es_a.enter_context(ExitStack())
        pss = es_a12.enter_context(tc.tile_pool(name="pssA", bufs=2, space="PSUM"))
        psb = es_a12.enter_context(tc.tile_pool(name="psbA", bufs=4, space="PSUM"))
        pha = es_a.enter_context(tc.tile_pool(name="pha", bufs=1))
        str_a = es_a.enter_context(tc.tile_pool(name="stra", bufs=2))
        expp = es_a.enter_context(
            tc.tile_pool(name="expp", bufs=18 if mm_dt != f32 else 10))
        finp = es_a.enter_context(tc.tile_pool(name="finp", bufs=3))

        wqkv = pha.tile([128, FCH, 3 * 128], mm_dt, tag="wqkv")
        for o in range(FCH):
            nc.sync.dma_start(wqkv[:, o, :], wqkv_d[o])
        augw = pha.tile([2, 3 * 128], mm_dt, tag="augw")
        nc.sync.dma_start(augw[:], augw_d[:])
        masks = pha.tile([128, 4, TT], att_dt, tag="masks")
        for kk in range(4):
            nc.sync.dma_start(masks[:, kk, :], mask_d[kk])

        aug_all = pha.tile([2, BT], mm_dt, tag="aug_all")
        r_ball = pha.tile([128, BT], mm_dt, tag="r_ball")
        qT = pha.tile([128, BT], att_dt, tag="qT")
        kT = pha.tile([128, BT], att_dt, tag="kT")
        v_slab = pha.tile([128, SCH, 130], att_dt, tag="v_slab")
        nc.vector.memset(v_slab[:], 1.0)
        r_cols = pha.tile([128, SCH], f32, tag="r_cols")

        # resident bf16 x slab (stats always; matmul rhs in bf16 mode)
        xhs = pha.tile([128, FCH, BT], mybir.dt.bfloat16, tag="xhs")
        for tj in range(NTT):
            for o in range(FCH):
                nc.sync.dma_start(xhs[:, o, tj * TT:(tj + 1) * TT],
                                  xh_d[o, :, tj * TT:(tj + 1) * TT])

        # ---- A1: all token stats (incl. r/sigma columns), pipelined
        for tj in range(NTT):
            tsl = slice(tj * TT, (tj + 1) * TT)
            ps_sum = pss.tile([1, TT], f32, tag="small")
            ps_sq = pss.tile([1, TT], f32, tag="small")
            for o in range(FCH):
                sq = str_a.tile([128, TT], mybir.dt.bfloat16, tag="sq")
                nc.vector.tensor_mul(sq[:], xhs[:, o, tsl], xhs[:, o, tsl])
                nc.tensor.matmul(ps_sum[:], ones_bf[:], xhs[:, o, tsl],
                                 start=(o == 0), stop=(o == FCH - 1))
                nc.tensor.matmul(ps_sq[:], ones_bf[:], sq[:],
                                 start=(o == 0), stop=(o == FCH - 1))
            mu_row = str_a.tile([1, TT], f32, tag="mu_row")
            nc.vector.tensor_scalar_mul(mu_row[:], ps_sum[:], 1.0 / E)
            msq_row = str_a.tile([1, TT], f32, tag="msq_row")
            nc.vector.tensor_scalar_mul(msq_row[:], ps_sq[:], 1.0 / E)
            var_row = str_a.tile([1, TT], f32, tag="var_row")
            nc.vector.tensor_mul(var_row[:], mu_row[:], mu_row[:])
            nc.vector.tensor_sub(var_row[:], msq_row[:], var_row[:])
            sig_row = str_a.tile([1, TT], f32, tag="sig_row")
            nc.scalar.activation(sig_row[:], var_row[:], AF.Sqrt, bias=eps1[:])
            r_row = str_a.tile([1, TT], f32, tag="r_row")
            nc.vector.reciprocal(r_row[:], sig_row[:])
            nc.vector.tensor_copy(aug_all[0:1, tsl], mu_row[:])
            nc.gpsimd.dma_start(aug_all[1:2, tsl], sig_row[:])
            r_row_mm = str_a.tile([1, TT], mm_dt, tag="r_row_mm")
            nc.vector.tensor_copy(r_row_mm[:], r_row[:])
            nc.gpsimd.partition_broadcast(r_ball[:, tsl], r_row_mm[:])
            for j in range(4):
                g = tj * 4 + j
                csl = slice(j * 128, (j + 1) * 128)
                pr = pss.tile([128, 1], f32, tag="small")
                nc.tensor.matmul(pr[:], r_row[0:1, csl], ones_f1[:])
                nc.vector.tensor_copy(r_cols[:, g:g + 1], pr[:])

        # ---- A2: dense QKV
        for tj in range(NTT):
            tsl = slice(tj * TT, (tj + 1) * TT)
            if use_f32_x:
                xt = str_a.tile([128, FCH, TT], f32, tag="xt")
                for o in range(FCH):
                    nc.sync.dma_start(xt[:, o, :], xT_d[o, :, tsl])
                x_mm = xt
                xsl = slice(0, TT)
            else:
                x_mm = xhs
                xsl = tsl
            for fg, slab in ((0, qT), (1, kT)):
                fsl = slice(fg * 128, (fg + 1) * 128)
                ps = psb.tile([128, TT], f32, tag="big")
                for o in range(FCH):
                    nc.tensor.matmul(ps[:], wqkv[:, o, fsl], x_mm[:, o, xsl],
                                     start=(o == 0), stop=False)
                nc.tensor.matmul(ps[:], augw[:, fsl], aug_all[:, tsl],
                                 start=False, stop=True)
                nc.vector.tensor_mul(slab[:, tsl], ps[:], r_ball[:, tsl])
            for j in range(4):
                g = tj * 4 + j
                csl = slice(xsl.start + j * 128, xsl.start + (j + 1) * 128)
                ps = psb.tile([128, 128], f32, tag="big")
                for o in range(FCH):
                    nc.tensor.matmul(ps[:], x_mm[:, o, csl],
                                     wqkv[:, o, 256:384],
                                     start=(o == 0), stop=False)
                nc.tensor.matmul(ps[:], aug_all[:, g * 128:(g + 1) * 128],
                                 augw[:, 256:384],
                                 start=False, stop=True)
                nc.vector.tensor_scalar_mul(v_slab[:, g, 0:64], ps[:, 0:64],
                                             r_cols[:, g:g + 1])
                nc.vector.tensor_scalar_mul(v_slab[:, g, 65:129], ps[:, 64:128],
                                            r_cols[:, g:g + 1])

        es_a12.close()
        psb = es_a.enter_context(tc.tile_pool(name="psbT", bufs=2, space="PSUM"))
        ps2 = es_a.enter_context(tc.tile_pool(name="ps2", bufs=2, space="PSUM"))

        # ---- A3: attention, software-pipelined one tile deep: in each
        # round, PV matmuls of the previous tile interleave with the score
        # matmuls of the current tile, so PE never waits on exp latency.
        def scores_thunks(hh, b, tj):
            hsl = slice(hh * 64, (hh + 1) * 64)
            tsl = slice(b * T + tj * TT, b * T + (tj + 1) * TT)
            n_si = 4 * tj + 4
            exs = []
            thunks = []
            state = {}

            def mk_mm(si, half):
                def f():
                    g = b * (T // 128) + si
                    ssl = slice(g * 128, (g + 1) * 128)
                    if half == 0:
                        state['ps'] = ps2.tile([128, 2, TT], f32, tag="big2", name="ps_pair")
                    ps = state['ps']
                    nc.tensor.matmul(ps[:, half, :], kT[hsl, ssl],
                                     qT[hsl, tsl], start=True, stop=True)
                return f

            def mk_exp(si_pair):
                def f():
                    ps = state['ps']
                    ex = expp.tile([128, 2, TT], att_dt, tag="ex")
                    nc.scalar.activation(ex[:], ps[:], AF.Exp,
                                         scale=float(E) ** -0.5)
                    for half, si in enumerate(si_pair):
                        exv = ex[:, half, :]
                        if si >= 4 * tj:
                            nc.vector.tensor_mul(exv, exv,
                                                 masks[:, si - 4 * tj, :])
                        g = b * (T // 128) + si
                        exs.append((g, exv))
                return f

            for si in range(0, n_si, 2):
                thunks.append(mk_mm(si, 0))
                thunks.append(mk_mm(si + 1, 1))
                thunks.append(mk_exp((si, si + 1)))
            return (hh, b, tj, exs, thunks)

        def pv_thunks(pend):
            hh, b, tj, exs, _ = pend
            hsl = slice(hh * 64, (hh + 1) * 64)
            tsl = slice(b * T + tj * TT, b * T + (tj + 1) * TT)
            vof = 65 * hh
            n = 4 * tj + 4
            po = psb.tile([65, TT], f32, tag="big")
            thunks = []

            def mk(idx):
                def f():
                    g, ex = exs[idx]
                    nc.tensor.matmul(po[:], v_slab[:, g, vof:vof + 65],
                                     ex, start=(idx == 0),
                                     stop=(idx == n - 1))
                return f

            for idx in range(n):
                thunks.append(mk(idx))

            def fin():
                rd = finp.tile([1, TT], f32, tag="rd")
                nc.vector.reciprocal(rd[:], po[64:65, :])
                rb = finp.tile([64, TT], f32, tag="rb")
                nc.gpsimd.partition_broadcast(rb[:], rd[:])
                nc.vector.tensor_mul(attn_slab[hsl, tsl], po[0:64, :], rb[:])
                d = b * (T // TT) + tj
                nc.sync.dma_start(a2a_in[hh][d],
                                  attn_slab[hsl, d * TB:(d + 1) * TB])
            thunks.append(fin)
            return thunks

        for hh in (0, 1):
            pend = None
            for b in range(B):
                for tj in range(T // TT):
                    cur = scores_thunks(hh, b, tj)
                    pv = pv_thunks(pend) if pend is not None else []
                    sc = cur[4]
                    for i in range(max(len(pv), len(sc))):
                        if i < len(pv):
                            pv[i]()
                        if i < len(sc):
                            sc[i]()
                    pend = cur
            for t in pv_thunks(pend):
                t()
            nc.gpsimd.collective_compute(
                "AllToAll", mybir.AluOpType.bypass,
                ins=[a2a_in[hh].opt()], outs=[a2a_out[hh].opt()],
                replica_groups=[list(range(W))],
            )

        es_a.close()

        # ========================================================= PHASE B
        psb = es.enter_context(tc.tile_pool(name="psbB", bufs=4, space="PSUM"))
        pss = es.enter_context(tc.tile_pool(name="pssB", bufs=2, space="PSUM"))
        phb = es.enter_context(tc.tile_pool(name="phb", bufs=1))
        str_b = es.enter_context(tc.tile_pool(name="strb", bufs=2))

        atf = phb.tile([128, FCH, TB], mm_dt, tag="atf")
        for o in range(FCH):
            nc.sync.dma_start(atf[0:64, o, :], a2a_out[0][o])
            nc.sync.dma_start(atf[64:128, o, :], a2a_out[1][o])
        xtb = phb.tile([128, FCH, TB], f32, tag="xtb")
        for o in range(FCH):
            nc.sync.dma_start(xtb[:, o, :], xtb_d[o])
        bproj = phb.tile([128, FCH], f32, tag="bproj")
        nc.sync.dma_start(bproj[:], bproj_d[:])
        b2c = phb.tile([128, FCH], f32, tag="b2c")
        nc.sync.dma_start(b2c[:], b2_d[:])
        augw1 = phb.tile([2, M4E], mm_dt, tag="augw1")
        nc.sync.dma_start(augw1[:], augw1_d[:])

        # ---- proj + residual -> x2T
        x2T = phb.tile([128, FCH, TB], f32, tag="x2T")
        for et in range(FCH):
            esl = slice(et * 128, (et + 1) * 128)
            wp = str_b.tile([128, FCH, 128], mm_dt, tag="wp")
            nc.sync.dma_start(wp[:], wproj_d[et].rearrange("p (o c) -> p o c", c=128))
            ps = psb.tile([128, TB], f32, tag="big")
            for o in range(FCH):
                nc.tensor.matmul(ps[:], wp[:, o, :], atf[:, o, :],
                                 start=(o == 0), stop=(o == FCH - 1))
            nc.vector.scalar_tensor_tensor(
                x2T[:, et, :], ps[:], bproj[:, et:et + 1], xtb[:, et, :],
                mybir.AluOpType.add, mybir.AluOpType.add)

        # ---- LN2 stats
        xh2 = phb.tile([128, FCH, TB], mybir.dt.bfloat16, tag="xh2")
        ps_sum = pss.tile([1, TB], f32, tag="small")
        ps_sq = pss.tile([1, TB], f32, tag="small")
        for o in range(FCH):
            nc.vector.tensor_copy(xh2[:, o, :], x2T[:, o, :])
            sq = str_b.tile([128, TB], mybir.dt.bfloat16, tag="sq2")
            nc.vector.tensor_mul(sq[:], xh2[:, o, :], xh2[:, o, :])
            nc.tensor.matmul(ps_sum[:], ones_bf[:], xh2[:, o, :],
                             start=(o == 0), stop=(o == FCH - 1))
            nc.tensor.matmul(ps_sq[:], ones_bf[:], sq[:],
                             start=(o == 0), stop=(o == FCH - 1))
        mu2 = phb.tile([1, TB], f32, tag="mu2")
        nc.vector.tensor_scalar_mul(mu2[:], ps_sum[:], 1.0 / E)
        msq2 = phb.tile([1, TB], f32, tag="msq2")
        nc.vector.tensor_scalar_mul(msq2[:], ps_sq[:], 1.0 / E)
        var2 = phb.tile([1, TB], f32, tag="var2")
        nc.vector.tensor_mul(var2[:], mu2[:], mu2[:])
        nc.vector.tensor_sub(var2[:], msq2[:], var2[:])
        sig2 = phb.tile([1, TB], f32, tag="sig2")
        nc.scalar.activation(sig2[:], var2[:], AF.Sqrt, bias=eps1[:])
        r2 = phb.tile([1, TB], f32, tag="r2")
        nc.vector.reciprocal(r2[:], sig2[:])
        aug2 = phb.tile([2, TB], mm_dt, tag="aug2")
        nc.vector.tensor_copy(aug2[0:1, :], mu2[:])
        nc.gpsimd.dma_start(aug2[1:2, :], sig2[:])
        r2_b = phb.tile([128, TB], f32, tag="r2_b")
        nc.gpsimd.partition_broadcast(r2_b[:], r2[:])

        x2mm = x2T if use_f32_x else xh2

        # ---- FFN1 -> relu slab (r2 deferred to FFN2 output: r2>0)
        relu = phb.tile([128, MCH, TB], mm_dt, tag="relu")
        for st in range(FCH):
            w1t = str_b.tile([128, FCH, 512], mm_dt, tag="w1t")
            nc.sync.dma_start(w1t[:], w1_d[st].rearrange("p (o c) -> p o c", c=512))
            for mj in range(4):
                mt = st * 4 + mj
                msl = slice(mt * 128, (mt + 1) * 128)
                jsl = slice(mj * 128, (mj + 1) * 128)
                ps = psb.tile([128, TB], f32, tag="big")
                for o in range(FCH):
                    nc.tensor.matmul(ps[:], w1t[:, o, jsl], x2mm[:, o, :],
                                     start=(o == 0), stop=False)
                nc.tensor.matmul(ps[:], augw1[:, msl], aug2[:],
                                 start=False, stop=True)
                nc.scalar.activation(relu[:, mt, :], ps[:], AF.Relu)

        # ---- FFN2 + r2 + residual + b2 -> y
        for et in range(FCH):
            ps = psb.tile([128, TB], f32, tag="big")
            for qq in range(4):
                w2t = str_b.tile([128, MCH // 4, 128], mm_dt, tag="w2t")
                nc.sync.dma_start(
                    w2t[:],
                    w2_d[et, :, qq * (M4E // 4):(qq + 1) * (M4E // 4)]
                    .rearrange("p (m f) -> p m f", f=128))
                for mj in range(MCH // 4):
                    mc = qq * (MCH // 4) + mj
                    nc.tensor.matmul(ps[:], w2t[:, mj, :], relu[:, mc, :],
                                     start=(mc == 0), stop=(mc == MCH - 1))
            u = str_b.tile([128, TB], f32, tag="u")
            nc.vector.tensor_mul(u[:], ps[:], r2_b[:])
            yt = str_b.tile([128, TB], f32, tag="yt")
            nc.vector.scalar_tensor_tensor(
                yt[:], u[:], b2c[:, et:et + 1], x2T[:, et, :],
                mybir.AluOpType.add, mybir.AluOpType.add)
            nc.sync.dma_start(y_d[et], yt[:])


    nc.finalize()
    return nc


def _get_nc():
    key = (MM_DT_S, ATT_DT_S)
    if key in _CACHE:
        return _CACHE[key]
    from concourse import bacc
    import concourse.mybir as mybir

    f32 = mybir.dt.float32
    mm_dt = f32 if MM_DT_S == "f32" else mybir.dt.bfloat16
    att_dt = f32 if ATT_DT_S == "f32" else mybir.dt.bfloat16
    nc = bacc.Bacc("TRN2", target_bir_lowering=False, debug=False,
                   num_devices=W)
    _build(nc, mm_dt, att_dt, f32)
    _CACHE[key] = nc
    return nc


def _prep_inputs(x, wq, wk, wv, w_proj, b_proj, w1, b1, w2, b2, g1, be1, g2, be2):
    """Host-side sharding: returns in_maps (list of 8 dicts)."""
    import ml_dtypes

    bf16 = ml_dtypes.bfloat16
    mm_np = np.float32 if MM_DT_S == "f32" else bf16
    att_np = np.float32 if ATT_DT_S == "f32" else bf16

    xf = np.ascontiguousarray(x.reshape(BT, E).T)          # [E, BT]
    xT = xf.reshape(FCH, 128, BT)
    xh = xT.astype(bf16)

    # causal mask tiles for the 4 diagonal-crossing offsets
    mask = np.zeros((4, 128, TT), dtype=att_np)
    uu = np.arange(TT)[None, :]
    pp = np.arange(128)[:, None]
    for k in range(4):
        mask[k] = (pp <= uu - 128 * k).astype(att_np)

    # [et][p][(o, c128)]: wproj_l[et, p, o*128+c] = w_proj[o*128+p, et*128+c]
    wpr = w_proj.reshape(FCH, 128, FCH, 128)                # [o, p, et, c]
    wproj_l = np.ascontiguousarray(wpr.transpose(2, 1, 0, 3).reshape(FCH, 128, E)).astype(mm_np)
    bproj_l = np.ascontiguousarray(b_proj.reshape(FCH, 128).T)  # [128, FCH]

    w1s = (g2[:, None] * w1)                                # [E, 4E]
    # [s][p][(o, c512)]: w1_l[s, p, o*512+c] = w1s[o*128+p, s*512+c]
    w1r = w1s.reshape(FCH, 128, FCH, 512)                   # [o, p, s, c]
    w1_l = np.ascontiguousarray(w1r.transpose(2, 1, 0, 3).reshape(FCH, 128, M4E)).astype(mm_np)
    aug1 = np.stack([-w1s.sum(axis=0), be2 @ w1 + b1]).astype(mm_np)

    # w2 host layout: [et, p, (mc, f)] with w2_l[et, p, mc*128+f] = w2[mc*128+p, et*128+f]
    w2r = w2.reshape(MCH, 128, FCH, 128)                    # [mc, p, et, f]
    w2_l = np.ascontiguousarray(w2r.transpose(2, 1, 0, 3).reshape(FCH, 128, M4E)).astype(mm_np)
    b2_l = np.ascontiguousarray(b2.reshape(FCH, 128).T)

    in_maps = []
    for c in range(W):
        hsl = slice(HPC * c, HPC * (c + 1))
        wq_c = wq[hsl].transpose(1, 0, 2).reshape(E, 128)
        wk_c = wk[hsl].transpose(1, 0, 2).reshape(E, 128)
        wv_c = wv[hsl].transpose(1, 0, 2).reshape(E, 128)
        wqkv = np.concatenate([g1[:, None] * wq_c,
                               g1[:, None] * wk_c,
                               g1[:, None] * wv_c], axis=1)  # [E, 384]
        augw = np.stack([-wqkv.sum(axis=0),
                         np.concatenate([be1 @ wq_c, be1 @ wk_c, be1 @ wv_c])]
                        ).astype(mm_np)
        m = {
            "xh": xh,
            "xtb": np.ascontiguousarray(xT[:, :, TB * c:TB * (c + 1)]),
            "wqkv": np.ascontiguousarray(wqkv.reshape(FCH, 128, 384)).astype(mm_np),
            "augw": augw,
            "wproj": wproj_l,
            "bproj": np.ascontiguousarray(bproj_l),
            "w1": w1_l,
            "augw1": aug1,
            "w2": w2_l,
            "b2c": np.ascontiguousarray(b2_l),
            "mask": mask,
        }
        if MM_DT_S == "f32":
            m["xT"] = xT
        in_maps.append(m)
    return in_maps


def kernel(**inputs):
    from concourse.bass_utils import run_bass_kernel_spmd

    nc = _get_nc()
    in_maps = _prep_inputs(**{k: np.asarray(v) for k, v in inputs.items()})
    res = run_bass_kernel_spmd(nc, in_maps, list(range(W)))
    # gather: core c produced y = [FCH, 128, TB] = yT block for tokens [TB*c, TB*(c+1))
    out_T = np.concatenate([res.results[c]["y"].reshape(E, TB)
                            for c in range(W)], axis=1)      # [E, BT]
    return np.ascontiguousarray(out_T.T).reshape(B, T, E).astype(np.float32)


# revision 33
# speedup vs baseline: 1.0323x; 1.0107x over previous
"""Trainium2 Bass kernel for a dense pre-LN transformer block.

Reference computation (B=2, T=2048, E=1024, H=16, HS=64):
    h  = LN(x; g1, be1)
    q, k, v = per-head projections of h        (wq/wk/wv: [H, E, HS])
    att = causal softmax(q k^T / sqrt(E)) v    (per head)
    x2 = x + concat(att) @ w_proj + b_proj
    y  = x2 + relu(LN(x2; g2, be2) @ w1 + b1) @ w2 + b2

Distribution over 8 NeuronCores:
  - QKV + attention: tensor-parallel over heads (2 heads/core).
  - proj + FFN: data-parallel over token rows (512 tokens/core).
  - One AllToAll (attn output, feature-major) glues the two.

Device-side layout is feature-major ("transposed"): activations are
[feature, token]; the host pre-transposes x and pre-arranges weights so
the kernel never transposes activations on device.

LayerNorm is folded into the following matmul:
    LN(x) @ W = r_t * (x @ (g*W) - mu_t * colsum(g*W) + sigma_t * (be@W))
applied via two rank-1 "augmentation" rows in the contraction, where
mu/sigma/r are per-token stats computed with ones-matmuls (bf16, exact
to ~1e-4). Softmax runs without max-subtraction (logits are bounded by
design: scale 1/sqrt(E)=1/32); the per-token r for V is folded into the
exp() bias as ln(r_s), and the softmax denominator is obtained from a
sigma-column appended to V in the PV matmul.
"""

import sys
from contextlib import ExitStack
import numpy as np

sys.path.insert(0, "/opt/trn_rl_repo")

# ---------------------------------------------------------------- constants
B, T, E, H = 2, 2048, 1024, 16
HS = E // H          # 64
W = 8                # cores
BT = B * T           # 4096 tokens total
TB = BT // W         # 512 tokens per core (row-DP block)
HPC = H // W         # 2 heads per core
FCH = E // 128       # 8 feature chunks
M4E = 4 * E          # 4096 ffn hidden
MCH = M4E // 128     # 32 hidden chunks
TT = 512             # token tile (matmul moving dim)
NTT = BT // TT       # 8 token tiles
SCH = BT // 128      # 32 token chunks of 128 (for v / s-chunks)
EPS = 1e-5

# dtype knobs: "f32" or "bf16"
import os as _os
MM_DT_S = _os.environ.get("KMM_DT", "bf16")    # linear-layer matmuls
ATT_DT_S = _os.environ.get("KATT_DT", "bf16")  # attention matmuls

_CACHE = {}


def _build(nc, mm_dt, att_dt, f32):
    import concourse.bass as bass
    from concourse.tile import TileContext
    import concourse.mybir as mybir

    AF = mybir.ActivationFunctionType
    dp = nc.declare_dram_parameter

    use_f32_x = mm_dt == f32  # stream fp32 x for linear matmuls

    # ------------------------------------------------- DRAM parameters
    if use_f32_x:
        xT_d = dp("xT", [FCH, 128, BT], f32, isOutput=False)
    xh_d = dp("xh", [FCH, 128, BT], mybir.dt.bfloat16, isOutput=False)
    xtb_d = dp("xtb", [FCH, 128, TB], f32, isOutput=False)
    wqkv_d = dp("wqkv", [FCH, 128, 3 * 128], mm_dt, isOutput=False)
    augw_d = dp("augw", [2, 3 * 128], mm_dt, isOutput=False)
    wproj_d = dp("wproj", [FCH, 128, E], mm_dt, isOutput=False)
    bproj_d = dp("bproj", [128, FCH], f32, isOutput=False)
    w1_d = dp("w1", [FCH, 128, M4E], mm_dt, isOutput=False)  # [s][p][(o,c512)] layout
    augw1_d = dp("augw1", [2, M4E], mm_dt, isOutput=False)
    w2_d = dp("w2", [FCH, 128, M4E], mm_dt, isOutput=False)  # [et,p,(mc,f)] host layout
    b2_d = dp("b2c", [128, FCH], f32, isOutput=False)
    mask_d = dp("mask", [4, 128, TT], att_dt, isOutput=False)
    y_d = dp("y", [FCH, 128, TB], f32, isOutput=True)


    es = ExitStack()
    with TileContext(nc) as tc, es:
        # ------------------------------------------------- pools
        glob = es.enter_context(tc.tile_pool(name="glob", bufs=1))
        dramp = es.enter_context(tc.tile_pool(name="dramp", bufs=1, space="DRAM"))
        a2a_in = [dramp.tile([W, 64, TB], mm_dt, tag=f"a2a_in{h}", name=f"a2a_in{h}") for h in (0, 1)]
        a2a_out = [dramp.tile([W, 64, TB], mm_dt, tag=f"a2a_out{h}", name=f"a2a_out{h}") for h in (0, 1)]


        ones_bf = glob.tile([128, 1], mybir.dt.bfloat16, tag="ones_bf")
        nc.vector.memset(ones_bf[:], 1.0)
        ones_f1 = glob.tile([1, 1], f32, tag="ones_f1")
        nc.vector.memset(ones_f1[:], 1.0)

        eps1 = glob.tile([1, 1], f32, tag="eps1")
        nc.vector.memset(eps1[:], EPS)
        attn_slab = glob.tile([128, BT], mm_dt, tag="attn_slab")

        # ========================================================= PHASE A
        es_a = es.enter_context(ExitStack())
        es_a12 = es_a.enter_context(ExitStack())
        pss = es_a12.enter_context(tc.tile_pool(name="pssA", bufs=2, space="PSUM"))
        psb = es_a12.enter_context(tc.tile_pool(name="psbA", bufs=4, space="PSUM"))
        pha = es_a.enter_context(tc.tile_pool(name="pha", bufs=1))
        str_a = es_a.enter_context(tc.tile_pool(name="stra", bufs=2))
        expp = es_a.enter_context(
            tc.tile_pool(name="expp", bufs=18 if mm_dt != f32 else 10))
        finp = es_a.enter_context(tc.tile_pool(name="finp", bufs=3))

        wqkv = pha.tile([128, FCH, 3 * 128], mm_dt, tag="wqkv")
        for o in range(FCH):
            nc.sync.dma_start(wqkv[:, o, :], wqkv_d[o])
        augw = pha.tile([2, 3 * 128], mm_dt, tag="augw")
        nc.sync.dma_start(augw[:], augw_d[:])
        masks = pha.tile([128, 4, TT], att_dt, tag="masks")
        for kk in range(4):
            nc.sync.dma_start(masks[:, kk, :], mask_d[kk])

        aug_all = pha.tile([2, BT], mm_dt, tag="aug_all")
        r_ball = pha.tile([128, BT], mm_dt, tag="r_ball")
        qT = pha.tile([128, BT], att_dt, tag="qT")
        kT = pha.tile([128, BT], att_dt, tag="kT")
        v_slab = pha.tile([128, SCH, 130], att_dt, tag="v_slab")
        nc.vector.memset(v_slab[:], 1.0)
        r_cols = pha.tile([128, SCH], f32, tag="r_cols")

        # resident bf16 x slab (stats always; matmul rhs in bf16 mode)
        xhs = pha.tile([128, FCH, BT], mybir.dt.bfloat16, tag="xhs")
        for tj in range(NTT):
            for o in range(FCH):
                nc.sync.dma_start(xhs[:, o, tj * TT:(tj + 1) * TT],
                                  xh_d[o, :, tj * TT:(tj + 1) * TT])

        def emit_stats(tj):
            tsl = slice(tj * TT, (tj + 1) * TT)
            ps_sum = pss.tile([1, TT], f32, tag="small")
            ps_sq = pss.tile([1, TT], f32, tag="small")
            for o in range(FCH):
                sq = str_a.tile([128, TT], mybir.dt.bfloat16, tag="sq")
                nc.vector.tensor_mul(sq[:], xhs[:, o, tsl], xhs[:, o, tsl])
                nc.tensor.matmul(ps_sum[:], ones_bf[:], xhs[:, o, tsl],
                                 start=(o == 0), stop=(o == FCH - 1))
                nc.tensor.matmul(ps_sq[:], ones_bf[:], sq[:],
                                 start=(o == 0), stop=(o == FCH - 1))
            mu_row = str_a.tile([1, TT], f32, tag="mu_row")
            nc.vector.tensor_scalar_mul(mu_row[:], ps_sum[:], 1.0 / E)
            msq_row = str_a.tile([1, TT], f32, tag="msq_row")
            nc.vector.tensor_scalar_mul(msq_row[:], ps_sq[:], 1.0 / E)
            var_row = str_a.tile([1, TT], f32, tag="var_row")
            nc.vector.tensor_mul(var_row[:], mu_row[:], mu_row[:])
            nc.vector.tensor_sub(var_row[:], msq_row[:], var_row[:])
            sig_row = str_a.tile([1, TT], f32, tag="sig_row")
            nc.scalar.activation(sig_row[:], var_row[:], AF.Sqrt, bias=eps1[:])
            r_row = str_a.tile([1, TT], f32, tag="r_row")
            nc.vector.reciprocal(r_row[:], sig_row[:])
            nc.vector.tensor_copy(aug_all[0:1, tsl], mu_row[:])
            nc.gpsimd.dma_start(aug_all[1:2, tsl], sig_row[:])
            r_row_mm = str_a.tile([1, TT], mm_dt, tag="r_row_mm")
            nc.vector.tensor_copy(r_row_mm[:], r_row[:])
            nc.gpsimd.partition_broadcast(r_ball[:, tsl], r_row_mm[:])
            for j in range(4):
                g = tj * 4 + j
                csl = slice(j * 128, (j + 1) * 128)
                pr = pss.tile([128, 1], f32, tag="small")
                nc.tensor.matmul(pr[:], r_row[0:1, csl], ones_f1[:])
                nc.vector.tensor_copy(r_cols[:, g:g + 1], pr[:])

        def emit_qkv(tj):
            tsl = slice(tj * TT, (tj + 1) * TT)
            for fg, slab in ((0, qT), (1, kT)):
                fsl = slice(fg * 128, (fg + 1) * 128)
                ps = psb.tile([128, TT], f32, tag="big")
                for o in range(FCH):
                    nc.tensor.matmul(ps[:], wqkv[:, o, fsl], xhs[:, o, tsl],
                                     start=(o == 0), stop=False)
                nc.tensor.matmul(ps[:], augw[:, fsl], aug_all[:, tsl],
                                 start=False, stop=True)
                nc.vector.tensor_mul(slab[:, tsl], ps[:], r_ball[:, tsl])
            for j in range(4):
                g = tj * 4 + j
                csl = slice(tj * TT + j * 128, tj * TT + (j + 1) * 128)
                ps = psb.tile([128, 128], f32, tag="big")
                for o in range(FCH):
                    nc.tensor.matmul(ps[:], xhs[:, o, csl],
                                     wqkv[:, o, 256:384],
                                     start=(o == 0), stop=False)
                nc.tensor.matmul(ps[:], aug_all[:, g * 128:(g + 1) * 128],
                                 augw[:, 256:384],
                                 start=False, stop=True)
                nc.vector.tensor_scalar_mul(v_slab[:, g, 0:64], ps[:, 0:64],
                                             r_cols[:, g:g + 1])
                nc.vector.tensor_scalar_mul(v_slab[:, g, 65:129], ps[:, 64:128],
                                            r_cols[:, g:g + 1])

        for tj in range(NTT):
            emit_stats(tj)
            if tj > 0:
                emit_qkv(tj - 1)
        emit_qkv(NTT - 1)

        es_a12.close()
        psb = es_a.enter_context(tc.tile_pool(name="psbT", bufs=2, space="PSUM"))
        ps2 = es_a.enter_context(tc.tile_pool(name="ps2", bufs=3, space="PSUM"))

        # ---- A3: attention, software-pipelined one tile deep: in each
        # round, PV matmuls of the previous tile interleave with the score
        # matmuls of the current tile, so PE never waits on exp latency.
        def scores_thunks(hh, b, tj):
            hsl = slice(hh * 64, (hh + 1) * 64)
            tsl = slice(b * T + tj * TT, b * T + (tj + 1) * TT)
            n_si = 4 * tj + 4
            exs = []
            thunks = []
            state = {}

            def mk_mm(si, half):
                def f():
                    g = b * (T // 128) + si
                    ssl = slice(g * 128, (g + 1) * 128)
                    if half == 0:
                        state['ps'] = ps2.tile([128, 2, TT], f32, tag="big2", name="ps_pair")
                    ps = state['ps']
                    nc.tensor.matmul(ps[:, half, :], kT[hsl, ssl],
                                     qT[hsl, tsl], start=True, stop=True)
                return f

            def mk_exp(si_pair):
                def f():
                    ps = state['ps']
                    ex = expp.tile([128, 2, TT], att_dt, tag="ex")
                    nc.scalar.activation(ex[:], ps[:], AF.Exp,
                                         scale=float(E) ** -0.5)
                    for half, si in enumerate(si_pair):
                        exv = ex[:, half, :]
                        if si >= 4 * tj:
                            nc.vector.tensor_mul(exv, exv,
                                                 masks[:, si - 4 * tj, :])
                        g = b * (T // 128) + si
                        exs.append((g, exv))
                return f

            for si in range(0, n_si, 2):
                thunks.append(mk_mm(si, 0))
                thunks.append(mk_mm(si + 1, 1))
                thunks.append(mk_exp((si, si + 1)))
            return (hh, b, tj, exs, thunks)

        def pv_thunks(pend):
            hh, b, tj, exs, _ = pend
            hsl = slice(hh * 64, (hh + 1) * 64)
            tsl = slice(b * T + tj * TT, b * T + (tj + 1) * TT)
            vof = 65 * hh
            n = 4 * tj + 4
            po = psb.tile([65, TT], f32, tag="big")
            thunks = []

            def mk(idx):
                def f():
                    g, ex = exs[idx]
                    nc.tensor.matmul(po[:], v_slab[:, g, vof:vof + 65],
                                     ex, start=(idx == 0),
                                     stop=(idx == n - 1))
                return f

            for idx in range(n):
                thunks.append(mk(idx))

            def fin():
                rd = finp.tile([1, TT], f32, tag="rd")
                nc.vector.reciprocal(rd[:], po[64:65, :])
                rb = finp.tile([64, TT], f32, tag="rb")
                nc.gpsimd.partition_broadcast(rb[:], rd[:])
                nc.vector.tensor_mul(attn_slab[hsl, tsl], po[0:64, :], rb[:])
                d = b * (T // TT) + tj
                nc.sync.dma_start(a2a_in[hh][d],
                                  attn_slab[hsl, d * TB:(d + 1) * TB])
            thunks.append(fin)
            return thunks

        for hh in (0, 1):
            pend = None
            for b in range(B):
                for tj in range(T // TT):
                    cur = scores_thunks(hh, b, tj)
                    pv = pv_thunks(pend) if pend is not None else []
                    sc = cur[4]
                    for i in range(max(len(pv), len(sc))):
                        if i < len(pv):
                            pv[i]()
                        if i < len(sc):
                            sc[i]()
                    pend = cur
            for t in pv_thunks(pend):
                t()
            nc.gpsimd.collective_compute(
                "AllToAll", mybir.AluOpType.bypass,
                ins=[a2a_in[hh].opt()], outs=[a2a_out[hh].opt()],
                replica_groups=[list(range(W))],
            )

        es_a.close()

        # ========================================================= PHASE B
        psb = es.enter_context(tc.tile_pool(name="psbB", bufs=4, space="PSUM"))
        pss = es.enter_context(tc.tile_pool(name="pssB", bufs=2, space="PSUM"))
        phb = es.enter_context(tc.tile_pool(name="phb", bufs=1))
        str_b = es.enter_context(tc.tile_pool(name="strb", bufs=2))
        w2p = es.enter_context(tc.tile_pool(name="w2p", bufs=4))

        atf = phb.tile([128, FCH, TB], mm_dt, tag="atf")
        for o in range(FCH):
            nc.sync.dma_start(atf[0:64, o, :], a2a_out[0][o])
            nc.sync.dma_start(atf[64:128, o, :], a2a_out[1][o])
        xtb = phb.tile([128, FCH, TB], f32, tag="xtb")
        for o in range(FCH):
            nc.sync.dma_start(xtb[:, o, :], xtb_d[o])
        bproj = phb.tile([128, FCH], f32, tag="bproj")
        nc.sync.dma_start(bproj[:], bproj_d[:])
        b2c = phb.tile([128, FCH], f32, tag="b2c")
        nc.sync.dma_start(b2c[:], b2_d[:])
        augw1 = phb.tile([2, M4E], mm_dt, tag="augw1")
        nc.sync.dma_start(augw1[:], augw1_d[:])

        # ---- proj + residual -> x2T
        x2T = phb.tile([128, FCH, TB], f32, tag="x2T")
        for et in range(FCH):
            esl = slice(et * 128, (et + 1) * 128)
            wp = str_b.tile([128, FCH, 128], mm_dt, tag="wp")
            nc.sync.dma_start(wp[:], wproj_d[et].rearrange("p (o c) -> p o c", c=128))
            ps = psb.tile([128, TB], f32, tag="big")
            for o in range(FCH):
                nc.tensor.matmul(ps[:], wp[:, o, :], atf[:, o, :],
                                 start=(o == 0), stop=(o == FCH - 1))
            nc.vector.scalar_tensor_tensor(
                x2T[:, et, :], ps[:], bproj[:, et:et + 1], xtb[:, et, :],
                mybir.AluOpType.add, mybir.AluOpType.add)

        # ---- LN2 stats
        xh2 = phb.tile([128, FCH, TB], mybir.dt.bfloat16, tag="xh2")
        ps_sum = pss.tile([1, TB], f32, tag="small")
        ps_sq = pss.tile([1, TB], f32, tag="small")
        for o in range(FCH):
            nc.vector.tensor_copy(xh2[:, o, :], x2T[:, o, :])
            sq = str_b.tile([128, TB], mybir.dt.bfloat16, tag="sq2")
            nc.vector.tensor_mul(sq[:], xh2[:, o, :], xh2[:, o, :])
            nc.tensor.matmul(ps_sum[:], ones_bf[:], xh2[:, o, :],
                             start=(o == 0), stop=(o == FCH - 1))
            nc.tensor.matmul(ps_sq[:], ones_bf[:], sq[:],
                             start=(o == 0), stop=(o == FCH - 1))
        mu2 = phb.tile([1, TB], f32, tag="mu2")
        nc.vector.tensor_scalar_mul(mu2[:], ps_sum[:], 1.0 / E)
        msq2 = phb.tile([1, TB], f32, tag="msq2")
        nc.vector.tensor_scalar_mul(msq2[:], ps_sq[:], 1.0 / E)
        var2 = phb.tile([1, TB], f32, tag="var2")
        nc.vector.tensor_mul(var2[:], mu2[:], mu2[:])
        nc.vector.tensor_sub(var2[:], msq2[:], var2[:])
        sig2 = phb.tile([1, TB], f32, tag="sig2")
        nc.scalar.activation(sig2[:], var2[:], AF.Sqrt, bias=eps1[:])
        r2 = phb.tile([1, TB], f32, tag="r2")
        nc.vector.reciprocal(r2[:], sig2[:])
        aug2 = phb.tile([2, TB], mm_dt, tag="aug2")
        nc.vector.tensor_copy(aug2[0:1, :], mu2[:])
        nc.gpsimd.dma_start(aug2[1:2, :], sig2[:])
        r2_b = phb.tile([128, TB], f32, tag="r2_b")
        nc.gpsimd.partition_broadcast(r2_b[:], r2[:])

        x2mm = x2T if use_f32_x else xh2

        # ---- FFN1 -> relu slab (r2 deferred to FFN2 output: r2>0)
        relu = phb.tile([128, MCH, TB], mm_dt, tag="relu")
        for st in range(FCH):
            w1t = str_b.tile([128, FCH, 512], mm_dt, tag="w1t")
            nc.sync.dma_start(w1t[:], w1_d[st].rearrange("p (o c) -> p o c", c=512))
            for mj in range(4):
                mt = st * 4 + mj
                msl = slice(mt * 128, (mt + 1) * 128)
                jsl = slice(mj * 128, (mj + 1) * 128)
                ps = psb.tile([128, TB], f32, tag="big")
                for o in range(FCH):
                    nc.tensor.matmul(ps[:], w1t[:, o, jsl], x2mm[:, o, :],
                                     start=(o == 0), stop=False)
                nc.tensor.matmul(ps[:], augw1[:, msl], aug2[:],
                                 start=False, stop=True)
                nc.scalar.activation(relu[:, mt, :], ps[:], AF.Relu)

        # ---- FFN2 + r2 + residual + b2 -> y
        for et in range(FCH):
            ps = psb.tile([128, TB], f32, tag="big")
            for qq in range(4):
                w2t = w2p.tile([128, MCH // 4, 128], mm_dt, tag="w2t")
                nc.sync.dma_start(
                    w2t[:],
                    w2_d[et, :, qq * (M4E // 4):(qq + 1) * (M4E // 4)]
                    .rearrange("p (m f) -> p m f", f=128))
                for mj in range(MCH // 4):
                    mc = qq * (MCH // 4) + mj
                    nc.tensor.matmul(ps[:], w2t[:, mj, :], relu[:, mc, :],
                                     start=(mc == 0), stop=(mc == MCH - 1))
            u = str_b.tile([128, TB], f32, tag="u")
            nc.vector.tensor_mul(u[:], ps[:], r2_b[:])
            yt = str_b.tile([128, TB], f32, tag="yt")
            nc.vector.scalar_tensor_tensor(
                yt[:], u[:], b2c[:, et:et + 1], x2T[:, et, :],
                mybir.AluOpType.add, mybir.AluOpType.add)
            nc.sync.dma_start(y_d[et], yt[:])


    nc.finalize()
    return nc


def _get_nc():
    key = (MM_DT_S, ATT_DT_S)
    if key in _CACHE:
        return _CACHE[key]
    from concourse import bacc
    import concourse.mybir as mybir

    f32 = mybir.dt.float32
    mm_dt = f32 if MM_DT_S == "f32" else mybir.dt.bfloat16
    att_dt = f32 if ATT_DT_S == "f32" else mybir.dt.bfloat16
    nc = bacc.Bacc("TRN2", target_bir_lowering=False, debug=False,
                   num_devices=W)
    _build(nc, mm_dt, att_dt, f32)
    _CACHE[key] = nc
    return nc


def _prep_inputs(x, wq, wk, wv, w_proj, b_proj, w1, b1, w2, b2, g1, be1, g2, be2):
    """Host-side sharding: returns in_maps (list of 8 dicts)."""
    import ml_dtypes

    bf16 = ml_dtypes.bfloat16
    mm_np = np.float32 if MM_DT_S == "f32" else bf16
    att_np = np.float32 if ATT_DT_S == "f32" else bf16

    xf = np.ascontiguousarray(x.reshape(BT, E).T)          # [E, BT]
    xT = xf.reshape(FCH, 128, BT)
    xh = xT.astype(bf16)

    # causal mask tiles for the 4 diagonal-crossing offsets
    mask = np.zeros((4, 128, TT), dtype=att_np)
    uu = np.arange(TT)[None, :]
    pp = np.arange(128)[:, None]
    for k in range(4):
        mask[k] = (pp <= uu - 128 * k).astype(att_np)

    # [et][p][(o, c128)]: wproj_l[et, p, o*128+c] = w_proj[o*128+p, et*128+c]
    wpr = w_proj.reshape(FCH, 128, FCH, 128)                # [o, p, et, c]
    wproj_l = np.ascontiguousarray(wpr.transpose(2, 1, 0, 3).reshape(FCH, 128, E)).astype(mm_np)
    bproj_l = np.ascontiguousarray(b_proj.reshape(FCH, 128).T)  # [128, FCH]

    w1s = (g2[:, None] * w1)                                # [E, 4E]
    # [s][p][(o, c512)]: w1_l[s, p, o*512+c] = w1s[o*128+p, s*512+c]
    w1r = w1s.reshape(FCH, 128, FCH, 512)                   # [o, p, s, c]
    w1_l = np.ascontiguousarray(w1r.transpose(2, 1, 0, 3).reshape(FCH, 128, M4E)).astype(mm_np)
    aug1 = np.stack([-w1s.sum(axis=0), be2 @ w1 + b1]).astype(mm_np)

    # w2 host layout: [et, p, (mc, f)] with w2_l[et, p, mc*128+f] = w2[mc*128+p, et*128+f]
    w2r = w2.reshape(MCH, 128, FCH, 128)                    # [mc, p, et, f]
    w2_l = np.ascontiguousarray(w2r.transpose(2, 1, 0, 3).reshape(FCH, 128, M4E)).astype(mm_np)
    b2_l = np.ascontiguousarray(b2.reshape(FCH, 128).T)

    in_maps = []
    for c in range(W):
        hsl = slice(HPC * c, HPC * (c + 1))
        wq_c = wq[hsl].transpose(1, 0, 2).reshape(E, 128)
        wk_c = wk[hsl].transpose(1, 0, 2).reshape(E, 128)
        wv_c = wv[hsl].transpose(1, 0, 2).reshape(E, 128)
        wqkv = np.concatenate([g1[:, None] * wq_c,
                               g1[:, None] * wk_c,
                               g1[:, None] * wv_c], axis=1)  # [E, 384]
        augw = np.stack([-wqkv.sum(axis=0),
                         np.concatenate([be1 @ wq_c, be1 @ wk_c, be1 @ wv_c])]
                        ).astype(mm_np)
        m = {
            "xh": xh,
            "xtb": np.ascontiguousarray(xT[:, :, TB * c:TB * (c + 1)]),
            "wqkv": np.ascontiguousarray(wqkv.reshape(FCH, 128, 384)).astype(mm_np),
            "augw": augw,
            "wproj": wproj_l,
            "bproj": np.ascontiguousarray(bproj_l),
            "w1": w1_l,
            "augw1": aug1,
            "w2": w2_l,
            "b2c": np.ascontiguousarray(b2_l),
            "mask": mask,
        }
        if MM_DT_S == "f32":
            m["xT"] = xT
        in_maps.append(m)
    return in_maps


def kernel(**inputs):
    from concourse.bass_utils import run_bass_kernel_spmd

    nc = _get_nc()
    in_maps = _prep_inputs(**{k: np.asarray(v) for k, v in inputs.items()})
    res = run_bass_kernel_spmd(nc, in_maps, list(range(W)))
    # gather: core c produced y = [FCH, 128, TB] = yT block for tokens [TB*c, TB*(c+1))
    out_T = np.concatenate([res.results[c]["y"].reshape(E, TB)
                            for c in range(W)], axis=1)      # [E, BT]
    return np.ascontiguousarray(out_T.T).reshape(B, T, E).astype(np.float32)
